# revision 8
# baseline (speedup 1.0000x reference)
"""Trainium2 Bass kernel for nn_ContrastiveLoss (SimCLR + spatial contrastive loss).

Strategy (8-core data parallel):
  - Host: L2-normalize z1/z2/embeddings, scale by 16 and quantize to fp8e4m3,
    build transposed [128,2,cols] operand tables, gather anchor rows, compute
    fp64 positive-pair dots.
  - Device (per core): fp8 DoubleRow matmuls (K=256 per instruction, 0.5
    cyc/col) of its 1024 simclr rows and 512 spatial rows against the full
    8192-column tables. The exp+row-sum drain of PSUM is split across two
    engines by row-tile:
      * ACT row-tiles: exact exp(x/T') via activation with accum_out.
      * DVE row-tiles: Schraudolph exp — one tensor_scalar affine writes the
        bf16 bit pattern of exp(x/T') as int16, a second bf16-view pass
        accumulates the row sum.
    A tiny fp8 Gram matmul per row-tile reproduces bit-exactly the main
    matmul's self-similarity diagonal; it is pushed through the SAME engine
    path as that row-tile, so the returned correction cancels the diagonal
    term exactly on the host.
  - Host: sum_exp = S_raw - corr, log, subtract positives, mean -> [2].

Self-contained: hardcodes shapes from the problem spec.
"""
import sys

for _p in ("/opt/trn_rl_repo", "/root/.axon_site/_ro/trn_rl_repo"):
    if _p not in sys.path:
        sys.path.insert(0, _p)

import numpy as np
import ml_dtypes

import concourse.tile as tile
from concourse import bacc, mybir
from concourse.bass_utils import run_bass_kernel_spmd

TEMPERATURE = 0.07
B = 4096     # simclr batch
D = 256      # projection dim
N = 8192     # num cells (spatial table rows, also 2B simclr table rows)
P = 4096     # num spatial pairs
NCORES = 8
SR = B // NCORES          # 512 simclr pair-rows per core (=> 1024 sim rows)
PR = P // NCORES          # 512 spatial rows per core
RT_SIMCLR = (2 * SR) // 128   # 8 row-tiles
RT_SPATIAL = PR // 128        # 4 row-tiles
RT_TOTAL = RT_SIMCLR + RT_SPATIAL  # 12
NGROUP = 4                # psum groups of 2048 columns
F32 = mybir.dt.float32
BF16 = mybir.dt.bfloat16
FP8 = mybir.dt.float8e4
I16 = mybir.dt.int16
MULT = mybir.AluOpType.mult
ADD = mybir.AluOpType.add
DR = mybir.MatmulPerfMode.DoubleRow

SQ = 16.0                                  # fp8 pre-quant scale per operand
SCALE_EFF = 1.0 / (TEMPERATURE * SQ * SQ)  # psum -> logit scale
# Schraudolph bf16-bit affine: bits = round(x*A + B); bitcast bf16 ~ exp(x*SCALE_EFF)
A_SCH = float(128.0 * np.log2(np.e) * SCALE_EFF)
B_SCH = float(16256.0 - 7.37)

# Whole-row-tile engine assignment (must match corr paths below):
DVE_RTS = frozenset({0, 1, 2, 10, 11})   # 3 simclr + 2 spatial row-tiles
# ACT: rts 3..9 (5 simclr + 2 spatial)

_CACHE = {}


def _build_nc():
    nc = bacc.Bacc("TRN2", target_bir_lowering=False)

    zT = nc.dram_tensor("zT", [128, 2, N], FP8, kind="ExternalInput")
    eT = nc.dram_tensor("eT", [128, 2, N], FP8, kind="ExternalInput")
    zTl = nc.dram_tensor("zTl", [128, 2, 2 * SR], FP8, kind="ExternalInput")
    aTl = nc.dram_tensor("aTl", [128, 2, PR], FP8, kind="ExternalInput")
    ident = nc.dram_tensor("ident", [128, 128], F32, kind="ExternalInput")

    sraw_o = nc.dram_tensor("sraw", [128, RT_TOTAL], F32, kind="ExternalOutput")
    corr_o = nc.dram_tensor("corr", [128, RT_TOTAL], F32, kind="ExternalOutput")

    with tile.TileContext(nc) as tc:
        with (
            tc.tile_pool(name="tabs", bufs=1) as tabs,
            tc.tile_pool(name="psA", bufs=2, space="PSUM") as psA,
            tc.tile_pool(name="psD", bufs=2, space="PSUM") as psD,
            tc.tile_pool(name="bits", bufs=1) as bitsp,
            tc.tile_pool(name="small", bufs=1) as small,
            tc.tile_pool(name="tmp", bufs=4) as tmpp,
        ):
            zTl_t = tabs.tile([128, 2, 2 * SR], FP8)
            aTl_t = tabs.tile([128, 2, PR], FP8)
            ident_t = small.tile([128, 128], F32)
            # First zT group in four 512-col sub-tiles so the first matmuls
            # only wait for a small DMA.
            zT_c = [tabs.tile([128, 2, 512], FP8, name=f"zTc{j}")
                    for j in range(4)]
            zT_g = [None] + [tabs.tile([128, 2, 2048], FP8, name=f"zTg{g}")
                             for g in range(1, NGROUP)]
            eT_g = [tabs.tile([128, 2, 2048], FP8, name=f"eTg{g}")
                    for g in range(NGROUP)]
            nc.sync.dma_start(zTl_t[:], zTl[:])
            nc.sync.dma_start(aTl_t[:], aTl[:])
            for j in range(4):
                nc.sync.dma_start(zT_c[j][:], zT[:, :, j * 512:(j + 1) * 512])
            nc.sync.dma_start(ident_t[:], ident[:])
            for g in range(1, NGROUP):
                nc.sync.dma_start(zT_g[g][:], zT[:, :, g * 2048:(g + 1) * 2048])
            for g in range(NGROUP):
                nc.sync.dma_start(eT_g[g][:], eT[:, :, g * 2048:(g + 1) * 2048])

            sraw_t = small.tile([128, RT_TOTAL], F32)
            corr_t = small.tile([128, RT_TOTAL], F32)

            def lhsT_of(rt):
                lh, li = (zTl_t, rt) if rt < RT_SIMCLR else (aTl_t, rt - RT_SIMCLR)
                return lh[:, :, li * 128:(li + 1) * 128]

            part_all = small.tile([128, RT_TOTAL, 8], F32)
            nc.vector.memset(part_all[:], 0.0)
            dve_rts = sorted(DVE_RTS)
            bits_rt = {rt: bitsp.tile([128, 8, 1024], I16, name=f"bits{rt}")
                       for rt in dve_rts}

            def emit_unit(rt, h):
                """2 DR matmuls + exp/sum of one (row-tile, 1024-col group).

                ACT and DVE row-tiles use separate 2-deep PSUM chains so each
                engine's next drain overlaps the other chain's refill."""
                lhsT = lhsT_of(rt)
                simclr = rt < RT_SIMCLR
                dve = rt in DVE_RTS
                pg = (psD if dve else psA).tile(
                    [128, 1024], F32, tag="bigD" if dve else "bigA")
                for cc in range(2):
                    j = h * 2 + cc
                    if simclr and j < 4:
                        rhs = zT_c[j][:]
                    else:
                        tab = zT_g[j // 4] if simclr else eT_g[j // 4]
                        rhs = tab[:, :, (j % 4) * 512:(j % 4 + 1) * 512]
                    nc.tensor.matmul(
                        pg[:, cc * 512:(cc + 1) * 512], lhsT, rhs,
                        start=True, stop=True, perf_mode=DR,
                    )
                if dve:
                    bt = bits_rt[rt]
                    nc.vector.tensor_scalar(bt[:, h, :], pg[:], A_SCH, B_SCH,
                                            MULT, ADD)
                    if h in (3, 7):
                        bv = bt[:, h - 3:h + 1, :].bitcast(BF16)
                        nc.vector.tensor_scalar(
                            bv, bv, 1.0, 0.0, MULT, ADD,
                            accum_out=part_all[:, rt, h // 4:h // 4 + 1])
                else:
                    # exp output is dead (only accum matters): write in place.
                    nc.scalar.activation(
                        pg[:], pg[:], mybir.ActivationFunctionType.Exp,
                        scale=SCALE_EFF,
                        accum_out=part_all[:, rt, h:h + 1])

            # Interleave ACT and DVE row-tiles within each column group so
            # both drain engines stay busy; simclr group-major so the first
            # arrived column groups feed all 8 row-tiles.
            def rr(rts):
                a = [rt for rt in rts if rt not in DVE_RTS]
                d = [rt for rt in rts if rt in DVE_RTS]
                out = []
                while a or d:
                    if a:
                        out.append(a.pop(0))
                    if d:
                        out.append(d.pop(0))
                return out

            for h in range(8):
                for rt in rr(range(RT_SIMCLR)):
                    emit_unit(rt, h)
            for h in range(8):
                for rt in rr(range(RT_SIMCLR, RT_TOTAL)):
                    emit_unit(rt, h)

            # Gram diagonals: diag(lhsT.T @ lhsT) is bitwise the main matmul's
            # self-similarity element for each row.
            pgrA = psA.tile([128, 1024], F32, tag="bigA")
            pgrD = psD.tile([128, 1024], F32, tag="bigD")
            for grt in range(RT_TOTAL):
                pgr_s = (pgrA[:, grt * 128:(grt + 1) * 128] if grt < 8 else
                         pgrD[:, (grt - 8) * 128:(grt - 7) * 128])
                nc.tensor.matmul(pgr_s, lhsT_of(grt), lhsT_of(grt),
                                 start=True, stop=True, perf_mode=DR)
            gd_all = tmpp.tile([128, RT_TOTAL, 128], F32, tag="gd")
            for grt in range(RT_TOTAL):
                pgr_s = (pgrA[:, grt * 128:(grt + 1) * 128] if grt < 8 else
                         pgrD[:, (grt - 8) * 128:(grt - 7) * 128])
                nc.vector.tensor_tensor(
                    gd_all[:, grt, :], pgr_s, ident_t[:], MULT,
                )
            gdv_all = tmpp.tile([128, RT_TOTAL], F32, tag="gdv")
            nc.vector.tensor_reduce(
                gdv_all[:], gd_all[:],
                axis=mybir.AxisListType.X, op=ADD,
            )
            # corr: per row-tile, through the SAME exp path as its main
            # units (deferred to overlap the tail of the unit stream).
            act_rts = [rt for rt in range(RT_TOTAL) if rt not in DVE_RTS]
            cbits = tmpp.tile([128, RT_TOTAL], I16, tag="cbits")
            for rt in act_rts:
                nc.scalar.activation(
                    corr_t[:, rt:rt + 1], gdv_all[:, rt:rt + 1],
                    mybir.ActivationFunctionType.Exp, scale=SCALE_EFF,
                )
            for rt in dve_rts:
                nc.vector.tensor_scalar(
                    cbits[:, rt:rt + 1], gdv_all[:, rt:rt + 1],
                    A_SCH, B_SCH, MULT, ADD,
                )
            cb_view = cbits[:].bitcast(BF16)
            for rt in dve_rts:
                nc.vector.tensor_scalar(
                    corr_t[:, rt:rt + 1], cb_view[:, rt:rt + 1],
                    1.0, None, MULT,
                )
            nc.sync.dma_start(corr_o[:], corr_t[:])

            nc.vector.tensor_reduce(
                sraw_t[:], part_all[:],
                axis=mybir.AxisListType.X, op=ADD,
            )
            nc.sync.dma_start(sraw_o[:], sraw_t[:])

    nc.finalize()
    return nc


def _l2norm(x):
    n = np.maximum(np.linalg.norm(x.astype(np.float32), axis=1, keepdims=True), 1e-12)
    return (x.astype(np.float32) / n).astype(np.float32)


def _pack_T(x):
    """[R, D=256] fp32 -> transposed fp8 operand table [128, 2, R] (x16)."""
    q = (x * np.float32(SQ)).astype(ml_dtypes.float8_e4m3)
    xT = np.ascontiguousarray(q.T)                      # [256, R]
    return np.ascontiguousarray(
        xT.reshape(2, 128, xT.shape[1]).transpose(1, 0, 2)
    )


def prepare(z1, z2, embeddings, anchor_idx, neighbor_idx):
    """Host-side prep: returns (in_maps, host_ctx)."""
    z1n = _l2norm(np.asarray(z1))
    z2n = _l2norm(np.asarray(z2))
    en = _l2norm(np.asarray(embeddings))
    ai = np.asarray(anchor_idx).astype(np.int64)
    ni = np.asarray(neighbor_idx).astype(np.int64)

    zcat = np.concatenate([z1n, z2n], axis=0)           # [2B, D]
    zT_p = _pack_T(zcat)                                # [128, 2, 8192] fp8
    eT_p = _pack_T(en)                                  # [128, 2, 8192] fp8
    a_rows = en[ai]                                     # [P, D] fp32
    aT_p = _pack_T(a_rows)                              # [128, 2, 4096] fp8

    # fp64 positive-pair logits (match reference semantics)
    psim = (np.sum(z1n.astype(np.float64) * z2n.astype(np.float64), axis=1)
            / np.float64(np.float32(TEMPERATURE)))      # [B]
    pos = (np.sum(a_rows.astype(np.float64) * en[ni].astype(np.float64), axis=1)
           / np.float64(np.float32(TEMPERATURE)))       # [P]
    eq = (ai == ni).astype(np.float64)                  # [P]

    ident = np.eye(128, dtype=np.float32)
    in_maps = []
    for c in range(NCORES):
        zTl_p = np.ascontiguousarray(np.concatenate(
            [zT_p[:, :, c * SR:(c + 1) * SR],
             zT_p[:, :, B + c * SR:B + (c + 1) * SR]], axis=2))  # [128,2,1024]
        aTl_p = np.ascontiguousarray(aT_p[:, :, c * PR:(c + 1) * PR])  # [128,2,512]
        in_maps.append({
            "zT": zT_p, "eT": eT_p, "zTl": zTl_p, "aTl": aTl_p, "ident": ident,
        })
    return in_maps, (psim, pos, eq)


def finish(results, host_ctx):
    """Host-side epilogue: assemble the two losses from per-core S_raw/corr."""
    psim, pos, eq = host_ctx
    terms1 = np.empty(2 * B, dtype=np.float64)
    terms2 = np.empty(P, dtype=np.float64)
    for c in range(NCORES):
        S = results[c]["sraw"].astype(np.float64).T.reshape(-1)   # [12*128]
        C = results[c]["corr"].astype(np.float64).T.reshape(-1)

        sum_exp = S[:2 * SR] - C[:2 * SR]
        p_loc = psim[c * SR:(c + 1) * SR]
        terms1[c * SR:(c + 1) * SR] = np.log(sum_exp[:SR]) - p_loc
        terms1[B + c * SR:B + (c + 1) * SR] = np.log(sum_exp[SR:2 * SR]) - p_loc

        s_sp = S[2 * SR:2 * SR + PR]
        c_sp = C[2 * SR:2 * SR + PR]
        g = slice(c * PR, (c + 1) * PR)
        total = s_sp - c_sp + eq[g] * np.exp(pos[g])
        terms2[g] = np.log(total) - pos[g]

    l1 = terms1.mean()
    l2 = terms2.mean()
    return np.array([l1, l2], dtype=np.float32)


def get_nc():
    if "nc" not in _CACHE:
        _CACHE["nc"] = _build_nc()
    return _CACHE["nc"]


def kernel(z1, z2, embeddings, anchor_idx, neighbor_idx):
    in_maps, host_ctx = prepare(z1, z2, embeddings, anchor_idx, neighbor_idx)
    nc = get_nc()
    res = run_bass_kernel_spmd(nc, in_maps, list(range(NCORES)))
    return finish(res.results, host_ctx)


# revision 9
# speedup vs baseline: 1.0272x; 1.0272x over previous
"""Trainium2 Bass kernel for nn_ContrastiveLoss (SimCLR + spatial contrastive loss).

Strategy (8-core data parallel):
  - Host: L2-normalize z1/z2/embeddings, scale by 16 and quantize to fp8e4m3,
    build transposed [128,2,cols] operand tables, gather anchor rows, compute
    fp64 positive-pair dots.
  - Device (per core): fp8 DoubleRow matmuls (K=256 per instruction, 0.5
    cyc/col) of its 1024 simclr rows and 512 spatial rows against the full
    8192-column tables. The exp+row-sum drain of PSUM is split across two
    engines by row-tile:
      * ACT row-tiles: exact exp(x/T') via activation with accum_out.
      * DVE row-tiles: Schraudolph exp — one tensor_scalar affine writes the
        bf16 bit pattern of exp(x/T') as int16, a second bf16-view pass
        accumulates the row sum.
    A tiny fp8 Gram matmul per row-tile reproduces bit-exactly the main
    matmul's self-similarity diagonal; it is pushed through the SAME engine
    path as that row-tile, so the returned correction cancels the diagonal
    term exactly on the host.
  - Host: sum_exp = S_raw - corr, log, subtract positives, mean -> [2].

Self-contained: hardcodes shapes from the problem spec.
"""
import sys

for _p in ("/opt/trn_rl_repo", "/root/.axon_site/_ro/trn_rl_repo"):
    if _p not in sys.path:
        sys.path.insert(0, _p)

import numpy as np
import ml_dtypes

import concourse.tile as tile
from concourse import bacc, mybir
from concourse.bass_utils import run_bass_kernel_spmd

TEMPERATURE = 0.07
B = 4096     # simclr batch
D = 256      # projection dim
N = 8192     # num cells (spatial table rows, also 2B simclr table rows)
P = 4096     # num spatial pairs
NCORES = 8
SR = B // NCORES          # 512 simclr pair-rows per core (=> 1024 sim rows)
PR = P // NCORES          # 512 spatial rows per core
RT_SIMCLR = (2 * SR) // 128   # 8 row-tiles
RT_SPATIAL = PR // 128        # 4 row-tiles
RT_TOTAL = RT_SIMCLR + RT_SPATIAL  # 12
NGROUP = 4                # psum groups of 2048 columns
F32 = mybir.dt.float32
BF16 = mybir.dt.bfloat16
FP8 = mybir.dt.float8e4
I16 = mybir.dt.int16
MULT = mybir.AluOpType.mult
ADD = mybir.AluOpType.add
DR = mybir.MatmulPerfMode.DoubleRow

SQ = 16.0                                  # fp8 pre-quant scale per operand
SCALE_EFF = 1.0 / (TEMPERATURE * SQ * SQ)  # psum -> logit scale
# Schraudolph bf16-bit affine: bits = round(x*A + B); bitcast bf16 ~ exp(x*SCALE_EFF)
A_SCH = float(128.0 * np.log2(np.e) * SCALE_EFF)
B_SCH = float(16256.0 - 7.37)

# Whole-row-tile engine assignment (must match corr paths below):
DVE_RTS = frozenset({0, 1, 2, 10, 11})   # 3 simclr + 2 spatial row-tiles
# ACT: rts 3..9 (5 simclr + 2 spatial)

_CACHE = {}


def _build_nc():
    nc = bacc.Bacc("TRN2", target_bir_lowering=False)

    zT = nc.dram_tensor("zT", [128, 2, N], FP8, kind="ExternalInput")
    eT = nc.dram_tensor("eT", [128, 2, N], FP8, kind="ExternalInput")
    zTl = nc.dram_tensor("zTl", [128, 2, 2 * SR], FP8, kind="ExternalInput")
    aTl = nc.dram_tensor("aTl", [128, 2, PR], FP8, kind="ExternalInput")
    ident = nc.dram_tensor("ident", [128, 128], F32, kind="ExternalInput")

    sraw_o = nc.dram_tensor("sraw", [128, RT_TOTAL], F32, kind="ExternalOutput")
    corr_o = nc.dram_tensor("corr", [128, RT_TOTAL], F32, kind="ExternalOutput")

    with tile.TileContext(nc) as tc:
        with (
            tc.tile_pool(name="tabs", bufs=1) as tabs,
            tc.tile_pool(name="psA", bufs=2, space="PSUM") as psA,
            tc.tile_pool(name="psD", bufs=2, space="PSUM") as psD,
            tc.tile_pool(name="bits", bufs=1) as bitsp,
            tc.tile_pool(name="small", bufs=1) as small,
            tc.tile_pool(name="tmp", bufs=4) as tmpp,
        ):
            zTl_t = tabs.tile([128, 2, 2 * SR], FP8)
            aTl_t = tabs.tile([128, 2, PR], FP8)
            ident_t = small.tile([128, 128], F32)
            # First zT group in four 512-col sub-tiles so the first matmuls
            # only wait for a small DMA.
            zT_c = [tabs.tile([128, 2, 512], FP8, name=f"zTc{j}")
                    for j in range(4)]
            zT_g = [None] + [tabs.tile([128, 2, 2048], FP8, name=f"zTg{g}")
                             for g in range(1, NGROUP)]
            eT_g = [tabs.tile([128, 2, 2048], FP8, name=f"eTg{g}")
                    for g in range(NGROUP)]
            nc.sync.dma_start(zTl_t[:], zTl[:])
            nc.sync.dma_start(aTl_t[:], aTl[:])
            for j in range(4):
                nc.sync.dma_start(zT_c[j][:], zT[:, :, j * 512:(j + 1) * 512])
            nc.sync.dma_start(ident_t[:], ident[:])
            nc.sync.dma_start(eT_g[0][:], eT[:, :, 0:2048])
            for g in range(1, NGROUP):
                nc.sync.dma_start(zT_g[g][:], zT[:, :, g * 2048:(g + 1) * 2048])
                nc.sync.dma_start(eT_g[g][:], eT[:, :, g * 2048:(g + 1) * 2048])

            sraw_t = small.tile([128, RT_TOTAL], F32)
            corr_t = small.tile([128, RT_TOTAL], F32)

            def lhsT_of(rt):
                lh, li = (zTl_t, rt) if rt < RT_SIMCLR else (aTl_t, rt - RT_SIMCLR)
                return lh[:, :, li * 128:(li + 1) * 128]

            # Gram diagonals: diag(lhsT.T @ lhsT) is bitwise the main matmul's
            # self-similarity element for each row.
            pgrA = psA.tile([128, 1024], F32, tag="bigA")
            pgrD = psD.tile([128, 1024], F32, tag="bigD")
            for grt in range(RT_TOTAL):
                pgr_s = (pgrA[:, grt * 128:(grt + 1) * 128] if grt < 8 else
                         pgrD[:, (grt - 8) * 128:(grt - 7) * 128])
                nc.tensor.matmul(pgr_s, lhsT_of(grt), lhsT_of(grt),
                                 start=True, stop=True, perf_mode=DR)
            gd_all = tmpp.tile([128, RT_TOTAL, 128], F32, tag="gd")
            for grt in range(RT_TOTAL):
                pgr_s = (pgrA[:, grt * 128:(grt + 1) * 128] if grt < 8 else
                         pgrD[:, (grt - 8) * 128:(grt - 7) * 128])
                nc.vector.tensor_tensor(
                    gd_all[:, grt, :], pgr_s, ident_t[:], MULT,
                )
            gdv_all = tmpp.tile([128, RT_TOTAL], F32, tag="gdv")
            nc.vector.tensor_reduce(
                gdv_all[:], gd_all[:],
                axis=mybir.AxisListType.X, op=ADD,
            )

            part_all = small.tile([128, RT_TOTAL, 8], F32)
            nc.vector.memset(part_all[:], 0.0)
            dve_rts = sorted(DVE_RTS)
            bits_rt = {rt: bitsp.tile([128, 8, 1024], I16, name=f"bits{rt}")
                       for rt in dve_rts}

            def emit_unit(rt, h):
                """2 DR matmuls + exp/sum of one (row-tile, 1024-col group).

                ACT and DVE row-tiles use separate 2-deep PSUM chains so each
                engine's next drain overlaps the other chain's refill."""
                lhsT = lhsT_of(rt)
                simclr = rt < RT_SIMCLR
                dve = rt in DVE_RTS
                pg = (psD if dve else psA).tile(
                    [128, 1024], F32, tag="bigD" if dve else "bigA")
                for cc in range(2):
                    j = h * 2 + cc
                    if simclr and j < 4:
                        rhs = zT_c[j][:]
                    else:
                        tab = zT_g[j // 4] if simclr else eT_g[j // 4]
                        rhs = tab[:, :, (j % 4) * 512:(j % 4 + 1) * 512]
                    nc.tensor.matmul(
                        pg[:, cc * 512:(cc + 1) * 512], lhsT, rhs,
                        start=True, stop=True, perf_mode=DR,
                    )
                if dve:
                    bt = bits_rt[rt]
                    nc.vector.tensor_scalar(bt[:, h, :], pg[:], A_SCH, B_SCH,
                                            MULT, ADD)
                    if h in (3, 7):
                        bv = bt[:, h - 3:h + 1, :].bitcast(BF16)
                        nc.vector.tensor_scalar(
                            bv, bv, 1.0, 0.0, MULT, ADD,
                            accum_out=part_all[:, rt, h // 4:h // 4 + 1])
                else:
                    # exp output is dead (only accum matters): write in place.
                    nc.scalar.activation(
                        pg[:], pg[:], mybir.ActivationFunctionType.Exp,
                        scale=SCALE_EFF,
                        accum_out=part_all[:, rt, h:h + 1])

            # Interleave ACT and DVE row-tiles within each column group so
            # both drain engines stay busy; simclr group-major so the first
            # arrived column groups feed all 8 row-tiles.
            def rr(rts):
                a = [rt for rt in rts if rt not in DVE_RTS]
                d = [rt for rt in rts if rt in DVE_RTS]
                out = []
                while a or d:
                    if a:
                        out.append(a.pop(0))
                    if d:
                        out.append(d.pop(0))
                return out

            for h in range(8):
                for rt in rr(range(RT_TOTAL)):
                    emit_unit(rt, h)

            # corr: per row-tile, through the SAME exp path as its main
            # units (deferred to overlap the tail of the unit stream).
            act_rts = [rt for rt in range(RT_TOTAL) if rt not in DVE_RTS]
            cbits = tmpp.tile([128, RT_TOTAL], I16, tag="cbits")
            for rt in act_rts:
                nc.scalar.activation(
                    corr_t[:, rt:rt + 1], gdv_all[:, rt:rt + 1],
                    mybir.ActivationFunctionType.Exp, scale=SCALE_EFF,
                )
            for rt in dve_rts:
                nc.vector.tensor_scalar(
                    cbits[:, rt:rt + 1], gdv_all[:, rt:rt + 1],
                    A_SCH, B_SCH, MULT, ADD,
                )
            cb_view = cbits[:].bitcast(BF16)
            for rt in dve_rts:
                nc.vector.tensor_scalar(
                    corr_t[:, rt:rt + 1], cb_view[:, rt:rt + 1],
                    1.0, None, MULT,
                )
            nc.sync.dma_start(corr_o[:], corr_t[:])

            nc.vector.tensor_reduce(
                sraw_t[:], part_all[:],
                axis=mybir.AxisListType.X, op=ADD,
            )
            nc.sync.dma_start(sraw_o[:], sraw_t[:])

    nc.finalize()
    return nc


def _l2norm(x):
    n = np.maximum(np.linalg.norm(x.astype(np.float32), axis=1, keepdims=True), 1e-12)
    return (x.astype(np.float32) / n).astype(np.float32)


def _pack_T(x):
    """[R, D=256] fp32 -> transposed fp8 operand table [128, 2, R] (x16)."""
    q = (x * np.float32(SQ)).astype(ml_dtypes.float8_e4m3)
    xT = np.ascontiguousarray(q.T)                      # [256, R]
    return np.ascontiguousarray(
        xT.reshape(2, 128, xT.shape[1]).transpose(1, 0, 2)
    )


def prepare(z1, z2, embeddings, anchor_idx, neighbor_idx):
    """Host-side prep: returns (in_maps, host_ctx)."""
    z1n = _l2norm(np.asarray(z1))
    z2n = _l2norm(np.asarray(z2))
    en = _l2norm(np.asarray(embeddings))
    ai = np.asarray(anchor_idx).astype(np.int64)
    ni = np.asarray(neighbor_idx).astype(np.int64)

    zcat = np.concatenate([z1n, z2n], axis=0)           # [2B, D]
    zT_p = _pack_T(zcat)                                # [128, 2, 8192] fp8
    eT_p = _pack_T(en)                                  # [128, 2, 8192] fp8
    a_rows = en[ai]                                     # [P, D] fp32
    aT_p = _pack_T(a_rows)                              # [128, 2, 4096] fp8

    # fp64 positive-pair logits (match reference semantics)
    psim = (np.sum(z1n.astype(np.float64) * z2n.astype(np.float64), axis=1)
            / np.float64(np.float32(TEMPERATURE)))      # [B]
    pos = (np.sum(a_rows.astype(np.float64) * en[ni].astype(np.float64), axis=1)
           / np.float64(np.float32(TEMPERATURE)))       # [P]
    eq = (ai == ni).astype(np.float64)                  # [P]

    ident = np.eye(128, dtype=np.float32)
    in_maps = []
    for c in range(NCORES):
        zTl_p = np.ascontiguousarray(np.concatenate(
            [zT_p[:, :, c * SR:(c + 1) * SR],
             zT_p[:, :, B + c * SR:B + (c + 1) * SR]], axis=2))  # [128,2,1024]
        aTl_p = np.ascontiguousarray(aT_p[:, :, c * PR:(c + 1) * PR])  # [128,2,512]
        in_maps.append({
            "zT": zT_p, "eT": eT_p, "zTl": zTl_p, "aTl": aTl_p, "ident": ident,
        })
    return in_maps, (psim, pos, eq)


def finish(results, host_ctx):
    """Host-side epilogue: assemble the two losses from per-core S_raw/corr."""
    psim, pos, eq = host_ctx
    terms1 = np.empty(2 * B, dtype=np.float64)
    terms2 = np.empty(P, dtype=np.float64)
    for c in range(NCORES):
        S = results[c]["sraw"].astype(np.float64).T.reshape(-1)   # [12*128]
        C = results[c]["corr"].astype(np.float64).T.reshape(-1)

        sum_exp = S[:2 * SR] - C[:2 * SR]
        p_loc = psim[c * SR:(c + 1) * SR]
        terms1[c * SR:(c + 1) * SR] = np.log(sum_exp[:SR]) - p_loc
        terms1[B + c * SR:B + (c + 1) * SR] = np.log(sum_exp[SR:2 * SR]) - p_loc

        s_sp = S[2 * SR:2 * SR + PR]
        c_sp = C[2 * SR:2 * SR + PR]
        g = slice(c * PR, (c + 1) * PR)
        total = s_sp - c_sp + eq[g] * np.exp(pos[g])
        terms2[g] = np.log(total) - pos[g]

    l1 = terms1.mean()
    l2 = terms2.mean()
    return np.array([l1, l2], dtype=np.float32)


def get_nc():
    if "nc" not in _CACHE:
        _CACHE["nc"] = _build_nc()
    return _CACHE["nc"]


def kernel(z1, z2, embeddings, anchor_idx, neighbor_idx):
    in_maps, host_ctx = prepare(z1, z2, embeddings, anchor_idx, neighbor_idx)
    nc = get_nc()
    res = run_bass_kernel_spmd(nc, in_maps, list(range(NCORES)))
    return finish(res.results, host_ctx)
